# revision 33
# baseline (speedup 1.0000x reference)
"""Trainium2 Bass kernel for nn_CrossAttention_34909494182275.

Cross-attention with the torch-reshape head split:
  Q = (x @ Wq.T + bq).reshape(NH, B, T, dh)   # row-major layout-mixing reshape
  scores = einsum('hbqd,hbkd', Q, K) / sqrt(dim_k)
  att = softmax(scores + adj)
  out = (einsum('hbqk,hbkd', att, V).reshape(B, T, dim_k)) @ Wo.T + bo

Slab decomposition: slab s = 16h+b of the head tensor is rows [256s, 256s+256)
of the flat [B*T, 1024] projection output; slab s uses adj[s % 16]; core c
handles slabs 8c..8c+7 -> x/y/out rows [2048c, +2048).  Perfectly
data-parallel across 8 cores, zero collectives.

Speedups over the fp16 baseline:
  * Q/K projections in fp8 (e4m3) with MatmulPerfMode.DoubleRow: the PE
    contracts 256/instruction at full rate -> ~1.9x faster than fp16.
    Measured end-to-end max-rel error ~1.56e-2 (budget 2e-2); V path / PV /
    out-proj stay fp16 (precision-critical).
  * bv/bo bias matmuls gone: softmax rows sum to 1, so att@(V+bv) = att@V+bv
    and out = dev_out + (bv@Wo.T + bo) is added on the HOST.  bq/bk fold into
    the Q/K PSUM evictions; 1/sqrt(dim_k) folds into the Q eviction scale.
  * q/k axes of the attention block processed in permuted order
    q' = 256*(t%4) + t//4 (same for k).  All strided evictions / matmul
    slices become contiguous; adj is host-permuted on both axes to match.
    The final output rows come out in natural order unchanged.
  * Out-projection of slab j runs as PE filler inside slab j+1's attention
    (fills the tail); eadj tiles prefetch across slab boundaries.
"""

import numpy as np

B, T, D = 16, 1024, 1024
NH, DH = 4, 256
NCORES = 8
NSLAB = 8  # slabs per core
NORM = 1.0 / 32.0  # 1/sqrt(1024)

_CACHE: dict = {}


def _build_program():
    from contextlib import ExitStack
    import functools

    import concourse.mybir as mybir
    import concourse.tile as tile
    from concourse import bacc
    from concourse.masks import make_identity

    fp8 = mybir.dt.float8e4
    fp16 = mybir.dt.float16
    f32 = mybir.dt.float32
    AF = mybir.ActivationFunctionType
    ALU = mybir.AluOpType
    DR = mybir.MatmulPerfMode.DoubleRow

    nc = bacc.Bacc("TRN2")
    x8_in = nc.dram_tensor("x8", [512, 2, 2048], fp8, kind="ExternalInput")
    y8_in = nc.dram_tensor("y8", [512, 2, 2048], fp8, kind="ExternalInput")
    yt_in = nc.dram_tensor("yt", [1024, 2048], fp16, kind="ExternalInput")
    eadj_in = nc.dram_tensor("eadj", [8, 1024, 1024], fp16, kind="ExternalInput")
    wq8_in = nc.dram_tensor("wq8", [512, 2, 1024], fp8, kind="ExternalInput")
    wk8_in = nc.dram_tensor("wk8", [512, 2, 1024], fp8, kind="ExternalInput")
    wvt_in = nc.dram_tensor("wvt", [1024, 1024], fp16, kind="ExternalInput")
    wot_in = nc.dram_tensor("wot", [1024, 1024], fp16, kind="ExternalInput")
    bqt_in = nc.dram_tensor("bqt", [128, 8], f32, kind="ExternalInput")
    bkt_in = nc.dram_tensor("bkt", [128, 8], f32, kind="ExternalInput")
    out_d = nc.dram_tensor("out", [2048, 1024], f32, kind="ExternalOutput")

    with tile.TileContext(nc) as tc, ExitStack() as ctx:
        singles = ctx.enter_context(tc.tile_pool(name="singles", bufs=1))
        wt = ctx.enter_context(tc.tile_pool(name="wt", bufs=1))
        # PSUM budget: 8 banks total.
        # ps_b16 (fp16 att-transpose batches, 2KB/part) x2 = 2 banks
        # ps_mm (fp32 matmul outs, <=2KB/part)          x2 = 2 banks
        # ps_sc (fp32 scores [128,1024], 4KB/part)      x2 = 4 banks
        ps_b16 = ctx.enter_context(tc.tile_pool(name="ps_b16", bufs=2, space="PSUM"))
        ps_mm = ctx.enter_context(tc.tile_pool(name="ps_mm", bufs=2, space="PSUM"))
        ps_sc = ctx.enter_context(tc.tile_pool(name="ps_sc", bufs=2, space="PSUM"))

        ident = singles.tile([128, 128], fp16)
        bqt = singles.tile([128, 8], f32)
        nc.sync.dma_start(out=bqt, in_=bqt_in[:])
        bkt = singles.tile([128, 8], f32)
        nc.sync.dma_start(out=bkt, in_=bkt_in[:])

        xt = ctx.enter_context(tc.tile_pool(name="xt", bufs=2))
        qkv = ctx.enter_context(tc.tile_pool(name="qkv", bufs=2))
        adjp = ctx.enter_context(tc.tile_pool(name="adjp", bufs=4))
        attp = ctx.enter_context(tc.tile_pool(name="attp", bufs=3))
        atp = ctx.enter_context(tc.tile_pool(name="atp", bufs=2))
        tmp = ctx.enter_context(tc.tile_pool(name="tmp", bufs=2))
        outp = ctx.enter_context(tc.tile_pool(name="outp", bufs=2))
        smalls = ctx.enter_context(tc.tile_pool(name="smalls", bufs=4))

        def emit_loads(j, first=False):
            """Per-slab activation loads.  fp8 pair tiles for Q/K projections
            (f = 128*(2*fp+i)+p), fp16 tiles for the V projection."""
            X8 = [
                xt.tile([128, 512], fp8, tag=f"x8_{fp}", name=f"x8_{fp}")
                for fp in range(4)
            ]
            Y8 = [
                xt.tile([128, 512], fp8, tag=f"y8_{fp}", name=f"y8_{fp}")
                for fp in range(4)
            ]
            Y16 = [
                xt.tile([128, 256], fp16, tag=f"y16_{fi}", name=f"y16_{fi}")
                for fi in range(8)
            ]
            # Steady state: hardware DGE queues (sync/scalar).  Prologue
            # (first=True): gpsimd, whose queue is independent of the weight
            # preload on sync/scalar, so slab-0 tiles land in parallel.
            engs = (
                (nc.gpsimd, nc.gpsimd) if first else (nc.sync, nc.scalar)
            )
            for fp in range(4):
                engs[fp % 2].dma_start(
                    out=X8[fp].rearrange("p (i n) -> p i n", i=2),
                    in_=x8_in[128 * fp : 128 * (fp + 1), :, 256 * j : 256 * (j + 1)],
                )
            for fp in range(4):
                engs[fp % 2].dma_start(
                    out=Y8[fp].rearrange("p (i n) -> p i n", i=2),
                    in_=y8_in[128 * fp : 128 * (fp + 1), :, 256 * j : 256 * (j + 1)],
                )
            for fi in range(8):
                engs[fi % 2].dma_start(
                    out=Y16[fi],
                    in_=yt_in[128 * fi : 128 * (fi + 1), 256 * j : 256 * (j + 1)],
                )
            return X8, Y8, Y16

        # ---- weights ----
        W8 = {
            w: [
                wt.tile([128, 2048], fp8, tag=f"w8_{w}_{fp}", name=f"w8_{w}_{fp}")
                for fp in range(4)
            ]
            for w in ("q", "k")
        }
        WT = {
            w: [
                wt.tile([128, 1024], fp16, tag=f"wt_{w}_{fi}", name=f"wt_{w}_{fi}")
                for fi in range(8)
            ]
            for w in ("v", "o")
        }

        # DMA priority order: Q-chain deps first (wq8 split across queues),
        # then slab-0 activations, wk8, eadj prefetch, wv, wo.
        for fp in range(4):
            eng = nc.sync if fp < 2 else nc.scalar
            eng.dma_start(
                out=W8["q"][fp].rearrange("p (i m) -> p i m", i=2),
                in_=wq8_in[128 * fp : 128 * (fp + 1)],
            )
        for fp in range(4):
            eng = nc.sync if fp < 2 else nc.scalar
            eng.dma_start(
                out=W8["k"][fp].rearrange("p (i m) -> p i m", i=2),
                in_=wk8_in[128 * fp : 128 * (fp + 1)],
            )
        XT0, YT0, Y160 = emit_loads(0, first=True)

        # rolling eadj prefetch (linear index a = 8*j + qt)
        eadj_tiles = {}

        def load_eadj(a):
            if a >= 64:
                return
            j, qt = a // 8, a % 8
            t = adjp.tile([128, 1024], fp16, tag="adj", name="eadj_t")
            nc.gpsimd.dma_start(out=t, in_=eadj_in[j, 128 * qt : 128 * (qt + 1), :])
            eadj_tiles[a] = t

        load_eadj(0)
        load_eadj(1)

        # wv/wo are first needed by the V(0) fillers (~28us) and out-proj(0)
        # (~50us); their loads are issued inside iteration j=0, behind
        # slab-1's tiles, keeping prologue DMA bandwidth for wq8/wk8/slab-0.
        # identity for PE transposes — needed only from attention-0 onward,
        # so emit after the prologue DMA issues
        make_identity(nc, ident)

        def proj_tasks(X8, Y8, Y16):
            """QsT/KsT/Vn tiles for a slab + 20 matmul-chain closures (PE
            filler work interleaved into the previous slab's attention)."""
            QsT = [
                qkv.tile([128, 1024], fp16, tag=f"q{d}", name=f"qst{d}")
                for d in range(2)
            ]
            KsT = [
                qkv.tile([128, 1024], fp16, tag=f"k{d}", name=f"kst{d}")
                for d in range(2)
            ]
            Vn = [
                qkv.tile([128, 1024], fp16, tag=f"v{nt}", name=f"vn{nt}")
                for nt in range(2)
            ]
            def qk_chain(TT8, W8l, bias_t, dst, kb, is_q):
                ps = ps_mm.tile([128, 256], f32, tag="pm", name="pmq")
                for fp in range(4):
                    nc.tensor.matmul(
                        ps,
                        W8l[fp].rearrange("p (i m) -> p i m", i=2)[
                            :, :, 128 * kb : 128 * (kb + 1)
                        ],
                        TT8[fp].rearrange("p (i n) -> p i n", i=2),
                        start=(fp == 0),
                        stop=(fp == 3),
                        perf_mode=DR,
                    )
                tm, dlo = kb // 2, kb % 2
                # permuted axis: q' = 256*tm + u -> contiguous eviction
                if is_q:
                    nc.vector.tensor_scalar(
                        out=dst[dlo][:, 256 * tm : 256 * (tm + 1)],
                        in0=ps,
                        scalar1=bias_t[:, kb : kb + 1],
                        scalar2=NORM,
                        op0=ALU.add,
                        op1=ALU.mult,
                    )
                else:
                    nc.vector.tensor_scalar(
                        out=dst[dlo][:, 256 * tm : 256 * (tm + 1)],
                        in0=ps,
                        scalar1=bias_t[:, kb : kb + 1],
                        scalar2=None,
                        op0=ALU.add,
                    )

            def v_chain(Y16l, Vdst, nt, kd):
                ps = ps_mm.tile([128, 512], f32, tag="pm", name="pmv")
                for fi in range(8):
                    nc.tensor.matmul(
                        ps,
                        Y16l[fi][:, 128 * nt : 128 * (nt + 1)],
                        WT["v"][fi][:, 512 * kd : 512 * (kd + 1)],
                        start=(fi == 0),
                        stop=(fi == 7),
                    )
                nc.scalar.copy(Vdst[nt][:, 512 * kd : 512 * (kd + 1)], ps)

            qtasks = [
                functools.partial(qk_chain, X8, W8["q"], bqt, QsT, kb, True)
                for kb in range(8)
            ]
            ktasks = [
                functools.partial(qk_chain, Y8, W8["k"], bkt, KsT, kb, False)
                for kb in range(8)
            ]
            vtasks = [
                functools.partial(v_chain, Y16, Vn, nt, kd)
                for nt in range(2)
                for kd in range(2)
            ]
            return QsT, KsT, Vn, qtasks + ktasks, vtasks

        def out_proj_tasks(j, TT_):
            """4 closures: out-proj chains for slab j, run as filler during
            slab j+1's attention.  ct==1 closures also evict + DMA."""
            osb = {}

            def chain(nt2, ct):
                if ct == 0:
                    osb[nt2] = outp.tile(
                        [128, 1024], f32, tag=f"o{nt2}", name=f"osb{nt2}"
                    )
                ps = ps_mm.tile([128, 512], f32, tag="pm")
                for g in range(8):
                    # permuted axis: contiguous lhsT slice
                    off = 256 * (g // 2) + 128 * nt2
                    nc.tensor.matmul(
                        ps,
                        TT_[g % 2][:, off : off + 128],
                        WT["o"][g][:, 512 * ct : 512 * (ct + 1)],
                        start=(g == 0),
                        stop=(g == 7),
                    )
                nc.scalar.copy(osb[nt2][:, 512 * ct : 512 * (ct + 1)], ps)
                if ct == 1:
                    nc.sync.dma_start(
                        out=out_d[
                            256 * j + 128 * nt2 : 256 * j + 128 * (nt2 + 1), :
                        ],
                        in_=osb[nt2],
                    )

            return [
                functools.partial(chain, nt2, ct) for nt2 in range(2) for ct in range(2)
            ]

        # prologue: slab 0 Q/K projections only (V(0) runs as attention-0
        # filler — V is first needed at PV).  K chains first: attention-0
        # needs ALL of KsT but only the first Q eviction per q'-tile.
        QsT, KsT, Vn, qk0, v0 = proj_tasks(XT0, YT0, Y160)
        for t in qk0[8:] + qk0[:8]:
            t()

        pending_v = v0  # V chains for the CURRENT slab
        pending_out = []  # out-proj filler from the previous slab
        for j in range(NSLAB):
            if j + 1 < NSLAB:
                XTn, YTn, Y16n = emit_loads(j + 1)
                Qn, Kn, Vv, qk_n, v_n = proj_tasks(XTn, YTn, Y16n)
            else:
                Qn = Kn = Vv = None
                qk_n, v_n = [], []
            if j == 0:
                for fi in range(8):
                    eng = nc.sync if fi % 2 == 0 else nc.scalar
                    eng.dma_start(
                        out=WT["v"][fi], in_=wvt_in[128 * fi : 128 * (fi + 1), :]
                    )
                for fi in range(8):
                    eng = nc.sync if fi % 2 == 0 else nc.scalar
                    eng.dma_start(
                        out=WT["o"][fi], in_=wot_in[128 * fi : 128 * (fi + 1), :]
                    )
            # filler queue: V(j), out-proj(j-1), Q/K(j+1) interleaved in 4
            # groups of (V, O, Q, K, Q, K)
            queue = []
            for i in range(4):
                if pending_v:
                    queue.append(pending_v.pop(0))
                if pending_out:
                    queue.append(pending_out.pop(0))
                queue.extend(qk_n[4 * i : 4 * (i + 1)])
            next_tasks = queue
            pending_v = v_n
            # front-loaded pops cover the pipeline ramp; the last slab has
            # only 8 filler tasks, spread 1/qt
            pops = [1] * 8 if j == NSLAB - 1 else [4, 4, 3, 3, 3, 3, 2, 2]

            # ---- attention, per q'-tile; filler interleaved ----
            # attT[p, 1024*blk + q'] = att^T[k'=128*blk+p, q']  (k'-block order;
            # blk -> (nt, tm) = (blk%2, blk//2))
            attT = atp.tile([128, 8192], fp16, tag="attT")
            attT3 = attT.rearrange("p (blk q) -> p blk q", blk=8)

            for qt in range(8):
                a = 8 * j + qt
                load_eadj(a + 2)
                eadj_t = eadj_tiles.pop(a)
                pss = ps_sc.tile([128, 1024], f32, tag="sc")
                for kh in range(2):
                    for dlo in range(2):
                        nc.tensor.matmul(
                            pss[:, 512 * kh : 512 * (kh + 1)],
                            QsT[dlo][:, 128 * qt : 128 * (qt + 1)],
                            KsT[dlo][:, 512 * kh : 512 * (kh + 1)],
                            start=(dlo == 0),
                            stop=(dlo == 1),
                        )
                exp_s = attp.tile([128, 1024], fp16, tag="exps")
                nc.scalar.activation(exp_s, pss, AF.Exp)
                attU = attp.tile([128, 1024], fp16, tag="attU")
                rsum = smalls.tile([128, 1], f32, tag="rsum")
                nc.vector.scalar_tensor_tensor(
                    out=attU,
                    in0=exp_s,
                    scalar=1.0,
                    in1=eadj_t,
                    op0=ALU.mult,
                    op1=ALU.mult,
                    accum_out=rsum,
                )
                recip = smalls.tile([128, 1], f32, tag="recip")
                nc.vector.reciprocal(recip, rsum)
                attN = attp.tile([128, 1024], fp16, tag="attN")
                nc.vector.tensor_scalar(
                    out=attN, in0=attU, scalar1=recip, scalar2=None, op0=ALU.mult
                )
                # PE filler while the softmax chain runs on ACT/DVE
                for _ in range(pops[qt]):
                    if next_tasks:
                        next_tasks.pop(0)()
                # attN -> attT: PE transposes of contiguous 128-col blocks
                # (k'-block order), then one batched PSUM->SBUF copy on DVE
                ps_at = ps_b16.tile([128, 1024], fp16, tag="pb")
                for blk in range(8):
                    src = attN[:, 128 * blk : 128 * (blk + 1)]
                    nc.tensor.transpose(ps_at[:, 128 * blk : 128 * (blk + 1)], src, ident)
                dst = attT3[:, :, 128 * qt : 128 * (qt + 1)]
                src3 = ps_at.rearrange("p (blk i) -> p blk i", blk=8)
                nc.vector.tensor_copy(dst, src3)

            # ---- PV: tempT[dlo][dv-128dlo, q'] ----
            TT_ = [tmp.tile([128, 1024], fp16, tag=f"tt{d}", name=f"tt{d}") for d in range(2)]
            for dlo in range(2):
                for qh in range(2):
                    ps = ps_mm.tile([128, 512], f32, tag="pm")
                    for blk in range(8):
                        nt, tm = blk % 2, blk // 2
                        nc.tensor.matmul(
                            ps,
                            Vn[nt][:, 256 * tm + 128 * dlo : 256 * tm + 128 * dlo + 128],
                            attT[:, 1024 * blk + 512 * qh : 1024 * blk + 512 * qh + 512],
                            start=(blk == 0),
                            stop=(blk == 7),
                        )
                    nc.scalar.copy(TT_[dlo][:, 512 * qh : 512 * (qh + 1)], ps)

            # drain any leftover filler (normally empty)
            while next_tasks:
                next_tasks.pop(0)()

            pending_out = out_proj_tasks(j, TT_)
            QsT, KsT, Vn = Qn, Kn, Vv

        # epilogue: last slab's out-projection
        for t in pending_out:
            t()

    nc.compile()
    return nc


def _get_program():
    if "nc" not in _CACHE:
        _CACHE["nc"] = _build_program()
    return _CACHE["nc"]


def _pair8(a):
    """[1024, n] -> fp8 pair-interleaved [512, 2, n]: out[128t+p, i, :] =
    a[128*(2t+i)+p, :]."""
    import ml_dtypes

    a8 = a.astype(ml_dtypes.float8_e4m3)
    n = a8.shape[1]
    return np.ascontiguousarray(
        a8.reshape(4, 2, 128, n).transpose(0, 2, 1, 3).reshape(512, 2, n)
    )


def _prep_inputs(x, y, adj, Wq, bq, Wk, bk, Wv, bv, Wo, bo):
    """Host-side prep: fp8/fp16 casts, transposes, exp(adj) with both axes
    permuted to q' = 256*(t%4) + t//4 order, per-core shards."""
    x2 = np.asarray(x, dtype=np.float32).reshape(B * T, D)
    y2 = np.asarray(y, dtype=np.float32).reshape(B * T, D)
    adj = np.asarray(adj, dtype=np.float32)

    xt32 = x2.T  # [1024, 16384]
    yt32 = y2.T
    yt16 = yt32.astype(np.float16)
    # exp(adj), both axes permuted t -> (t%4)*256 + t//4
    eadj16 = (
        np.exp(adj)
        .astype(np.float16)
        .reshape(16, 256, 4, 256, 4)
        .transpose(0, 2, 1, 4, 3)
        .reshape(16, 1024, 1024)
    )
    eadj16 = np.ascontiguousarray(eadj16)

    wq8 = _pair8(np.asarray(Wq, np.float32).T)  # unscaled; NORM folded in evict
    wk8 = _pair8(np.asarray(Wk, np.float32).T)
    wvt = np.asarray(Wv, np.float32).T.astype(np.float16)
    wot = np.asarray(Wo, np.float32).T.astype(np.float16)

    bqt = np.ascontiguousarray(np.asarray(bq, np.float32).reshape(8, 128).T)
    bkt = np.ascontiguousarray(np.asarray(bk, np.float32).reshape(8, 128).T)

    in_maps = []
    for c in range(NCORES):
        sl = slice(2048 * c, 2048 * (c + 1))
        in_maps.append(
            {
                "x8": _pair8(xt32[:, sl]),
                "y8": _pair8(yt32[:, sl]),
                "yt": np.ascontiguousarray(yt16[:, sl]),
                "eadj": eadj16[8 * (c % 2) : 8 * (c % 2) + 8],
                "wq8": wq8,
                "wk8": wk8,
                "wvt": wvt,
                "wot": wot,
                "bqt": bqt,
                "bkt": bkt,
            }
        )
    return in_maps


def kernel(x, y, adj, Wq, bq, Wk, bk, Wv, bv, Wo, bo):
    from concourse.bass_utils import run_bass_kernel_spmd

    nc = _get_program()
    in_maps = _prep_inputs(x, y, adj, Wq, bq, Wk, bk, Wv, bv, Wo, bo)
    res = run_bass_kernel_spmd(nc, in_maps, list(range(NCORES)))
    out = np.concatenate([res.results[c]["out"] for c in range(NCORES)], axis=0)
    # bv/bo fold: softmax rows sum to 1, so att@(V+bv) = att@V + bv and
    # out = dev_out + (bv @ Wo.T + bo)
    hb = np.asarray(bv, np.float32) @ np.asarray(Wo, np.float32).T + np.asarray(
        bo, np.float32
    )
    out = out + hb[None, :]
    return out.reshape(B, T, D)


# revision 35
# speedup vs baseline: 1.1686x; 1.1686x over previous
"""Trainium2 Bass kernel for nn_CrossAttention_34909494182275.

Cross-attention with the torch-reshape head split:
  Q = (x @ Wq.T + bq).reshape(NH, B, T, dh)   # row-major layout-mixing reshape
  scores = einsum('hbqd,hbkd', Q, K) / sqrt(dim_k)
  att = softmax(scores + adj)
  out = (einsum('hbqk,hbkd', att, V).reshape(B, T, dim_k)) @ Wo.T + bo

Slab decomposition: slab s = 16h+b of the head tensor is rows [256s, 256s+256)
of the flat [B*T, 1024] projection output; slab s uses adj[s % 16]; core c
handles slabs 8c..8c+7 -> x/y/out rows [2048c, +2048).  Perfectly
data-parallel across 8 cores, zero collectives.

Speedups over the fp16 baseline:
  * Q/K projections in fp8 (e4m3) with MatmulPerfMode.DoubleRow: the PE
    contracts 256/instruction at full rate -> ~1.9x faster than fp16.
    Measured end-to-end max-rel error ~1.56e-2 (budget 2e-2); V path / PV /
    out-proj stay fp16 (precision-critical).
  * bv/bo bias matmuls gone: softmax rows sum to 1, so att@(V+bv) = att@V+bv
    and out = dev_out + (bv@Wo.T + bo) is added on the HOST.  bq/bk fold into
    the Q/K PSUM evictions; 1/sqrt(dim_k) folds into the Q eviction scale.
  * q/k axes of the attention block processed in permuted order
    q' = 256*(t%4) + t//4 (same for k).  All strided evictions / matmul
    slices become contiguous; adj is host-permuted on both axes to match.
    The final output rows come out in natural order unchanged.
  * Out-projection of slab j runs as PE filler inside slab j+1's attention
    (fills the tail); eadj tiles prefetch across slab boundaries.
"""

import numpy as np

B, T, D = 16, 1024, 1024
NH, DH = 4, 256
NCORES = 8
NSLAB = 8  # slabs per core
NORM = 1.0 / 32.0  # 1/sqrt(1024)

_CACHE: dict = {}


def _build_program():
    from contextlib import ExitStack
    import functools

    import concourse.mybir as mybir
    import concourse.tile as tile
    from concourse import bacc
    from concourse.masks import make_identity

    fp8 = mybir.dt.float8e4
    fp16 = mybir.dt.float16
    f32 = mybir.dt.float32
    AF = mybir.ActivationFunctionType
    ALU = mybir.AluOpType
    DR = mybir.MatmulPerfMode.DoubleRow

    nc = bacc.Bacc("TRN2")
    x8_in = nc.dram_tensor("x8", [512, 2, 2048], fp8, kind="ExternalInput")
    y8_in = nc.dram_tensor("y8", [512, 2, 2048], fp8, kind="ExternalInput")
    yt_in = nc.dram_tensor("yt", [1024, 2048], fp16, kind="ExternalInput")
    eadj_in = nc.dram_tensor("eadj", [8, 1024, 1024], fp16, kind="ExternalInput")
    wq8_in = nc.dram_tensor("wq8", [512, 2, 1024], fp8, kind="ExternalInput")
    wk8_in = nc.dram_tensor("wk8", [512, 2, 1024], fp8, kind="ExternalInput")
    wvt_in = nc.dram_tensor("wvt", [1024, 1024], fp16, kind="ExternalInput")
    wot_in = nc.dram_tensor("wot", [1024, 1024], fp16, kind="ExternalInput")
    bqt_in = nc.dram_tensor("bqt", [128, 8], f32, kind="ExternalInput")
    bkt_in = nc.dram_tensor("bkt", [128, 8], f32, kind="ExternalInput")
    out_d = nc.dram_tensor("out", [2048, 1024], f32, kind="ExternalOutput")

    with tile.TileContext(nc) as tc, ExitStack() as ctx:
        singles = ctx.enter_context(tc.tile_pool(name="singles", bufs=1))
        wt = ctx.enter_context(tc.tile_pool(name="wt", bufs=1))
        # PSUM budget: 8 banks total.
        # ps_b16 (fp16 att-transpose batches, 2KB/part) x2 = 2 banks
        # ps_mm (fp32 matmul outs, <=2KB/part)          x2 = 2 banks
        # ps_sc (fp32 scores [128,1024], 4KB/part)      x2 = 4 banks
        ps_b16 = ctx.enter_context(tc.tile_pool(name="ps_b16", bufs=2, space="PSUM"))
        ps_mm = ctx.enter_context(tc.tile_pool(name="ps_mm", bufs=2, space="PSUM"))
        ps_sc = ctx.enter_context(tc.tile_pool(name="ps_sc", bufs=2, space="PSUM"))

        ident = singles.tile([128, 128], fp16)
        bqt = singles.tile([128, 8], f32)
        nc.sync.dma_start(out=bqt, in_=bqt_in[:])
        bkt = singles.tile([128, 8], f32)
        nc.sync.dma_start(out=bkt, in_=bkt_in[:])

        xt = ctx.enter_context(tc.tile_pool(name="xt", bufs=2))
        qkv = ctx.enter_context(tc.tile_pool(name="qkv", bufs=2))
        adjp = ctx.enter_context(tc.tile_pool(name="adjp", bufs=4))
        attp = ctx.enter_context(tc.tile_pool(name="attp", bufs=3))
        atp = ctx.enter_context(tc.tile_pool(name="atp", bufs=2))
        tmp = ctx.enter_context(tc.tile_pool(name="tmp", bufs=2))
        outp = ctx.enter_context(tc.tile_pool(name="outp", bufs=2))
        smalls = ctx.enter_context(tc.tile_pool(name="smalls", bufs=4))

        def emit_loads(j, first=False):
            """Per-slab activation loads.  fp8 pair tiles for Q/K projections
            (f = 128*(2*fp+i)+p), fp16 tiles for the V projection."""
            X8 = [
                xt.tile([128, 512], fp8, tag=f"x8_{fp}", name=f"x8_{fp}")
                for fp in range(4)
            ]
            Y8 = [
                xt.tile([128, 512], fp8, tag=f"y8_{fp}", name=f"y8_{fp}")
                for fp in range(4)
            ]
            Y16 = [
                xt.tile([128, 256], fp16, tag=f"y16_{fi}", name=f"y16_{fi}")
                for fi in range(8)
            ]
            # Steady state: hardware DGE queues (sync/scalar).  Prologue
            # (first=True): gpsimd, whose queue is independent of the weight
            # preload on sync/scalar, so slab-0 tiles land in parallel.
            engs = (
                (nc.gpsimd, nc.gpsimd) if first else (nc.sync, nc.scalar)
            )
            for fp in range(4):
                engs[fp % 2].dma_start(
                    out=X8[fp].rearrange("p (i n) -> p i n", i=2),
                    in_=x8_in[128 * fp : 128 * (fp + 1), :, 256 * j : 256 * (j + 1)],
                )
            for fp in range(4):
                engs[fp % 2].dma_start(
                    out=Y8[fp].rearrange("p (i n) -> p i n", i=2),
                    in_=y8_in[128 * fp : 128 * (fp + 1), :, 256 * j : 256 * (j + 1)],
                )
            for fi in range(8):
                engs[fi % 2].dma_start(
                    out=Y16[fi],
                    in_=yt_in[128 * fi : 128 * (fi + 1), 256 * j : 256 * (j + 1)],
                )
            return X8, Y8, Y16

        # ---- weights ----
        W8 = {
            w: [
                wt.tile([128, 2048], fp8, tag=f"w8_{w}_{fp}", name=f"w8_{w}_{fp}")
                for fp in range(4)
            ]
            for w in ("q", "k")
        }
        WT = {
            w: [
                wt.tile([128, 1024], fp16, tag=f"wt_{w}_{fi}", name=f"wt_{w}_{fi}")
                for fi in range(8)
            ]
            for w in ("v", "o")
        }

        # DMA priority order: Q-chain deps first (wq8 split across queues),
        # then slab-0 activations, wk8, eadj prefetch, wv, wo.
        for fp in range(4):
            eng = nc.sync if fp < 2 else nc.scalar
            eng.dma_start(
                out=W8["q"][fp].rearrange("p (i m) -> p i m", i=2),
                in_=wq8_in[128 * fp : 128 * (fp + 1)],
            )
        for fp in range(4):
            eng = nc.sync if fp < 2 else nc.scalar
            eng.dma_start(
                out=W8["k"][fp].rearrange("p (i m) -> p i m", i=2),
                in_=wk8_in[128 * fp : 128 * (fp + 1)],
            )
        XT0, YT0, Y160 = emit_loads(0, first=True)

        # rolling eadj prefetch (linear index a = 8*j + qt)
        eadj_tiles = {}

        def load_eadj(a):
            if a >= 64:
                return
            j, qt = a // 8, a % 8
            t = adjp.tile([128, 1024], fp16, tag="adj", name="eadj_t")
            nc.gpsimd.dma_start(out=t, in_=eadj_in[j, 128 * qt : 128 * (qt + 1), :])
            eadj_tiles[a] = t

        load_eadj(0)
        load_eadj(1)

        for fi in range(8):
            eng = nc.sync if fi % 2 == 0 else nc.scalar
            eng.dma_start(out=WT["v"][fi], in_=wvt_in[128 * fi : 128 * (fi + 1), :])
        # wo is first needed by out-proj(0) during attention-1 (~50us in);
        # its load is issued inside iteration j=0, behind slab-1's tiles
        # identity for PE transposes — needed only from attention-0 onward,
        # so emit after the prologue DMA issues
        make_identity(nc, ident)

        def proj_tasks(X8, Y8, Y16):
            """QsT/KsT/Vn tiles for a slab + 20 matmul-chain closures (PE
            filler work interleaved into the previous slab's attention)."""
            QsT = [
                qkv.tile([128, 1024], fp16, tag=f"q{d}", name=f"qst{d}")
                for d in range(2)
            ]
            KsT = [
                qkv.tile([128, 1024], fp16, tag=f"k{d}", name=f"kst{d}")
                for d in range(2)
            ]
            Vn = [
                qkv.tile([128, 1024], fp16, tag=f"v{nt}", name=f"vn{nt}")
                for nt in range(2)
            ]
            def qk_chain(TT8, W8l, bias_t, dst, kb, is_q):
                ps = ps_mm.tile([128, 256], f32, tag="pm", name="pmq")
                for fp in range(4):
                    nc.tensor.matmul(
                        ps,
                        W8l[fp].rearrange("p (i m) -> p i m", i=2)[
                            :, :, 128 * kb : 128 * (kb + 1)
                        ],
                        TT8[fp].rearrange("p (i n) -> p i n", i=2),
                        start=(fp == 0),
                        stop=(fp == 3),
                        perf_mode=DR,
                    )
                tm, dlo = kb // 2, kb % 2
                # permuted axis: q' = 256*tm + u -> contiguous eviction
                if is_q:
                    nc.vector.tensor_scalar(
                        out=dst[dlo][:, 256 * tm : 256 * (tm + 1)],
                        in0=ps,
                        scalar1=bias_t[:, kb : kb + 1],
                        scalar2=NORM,
                        op0=ALU.add,
                        op1=ALU.mult,
                    )
                else:
                    nc.vector.tensor_scalar(
                        out=dst[dlo][:, 256 * tm : 256 * (tm + 1)],
                        in0=ps,
                        scalar1=bias_t[:, kb : kb + 1],
                        scalar2=None,
                        op0=ALU.add,
                    )

            def v_chain(Y16l, Vdst, nt, kd):
                ps = ps_mm.tile([128, 512], f32, tag="pm", name="pmv")
                for fi in range(8):
                    nc.tensor.matmul(
                        ps,
                        Y16l[fi][:, 128 * nt : 128 * (nt + 1)],
                        WT["v"][fi][:, 512 * kd : 512 * (kd + 1)],
                        start=(fi == 0),
                        stop=(fi == 7),
                    )
                nc.scalar.copy(Vdst[nt][:, 512 * kd : 512 * (kd + 1)], ps)

            qtasks = [
                functools.partial(qk_chain, X8, W8["q"], bqt, QsT, kb, True)
                for kb in range(8)
            ]
            ktasks = [
                functools.partial(qk_chain, Y8, W8["k"], bkt, KsT, kb, False)
                for kb in range(8)
            ]
            vtasks = [
                functools.partial(v_chain, Y16, Vn, nt, kd)
                for nt in range(2)
                for kd in range(2)
            ]
            return QsT, KsT, Vn, qtasks + ktasks, vtasks

        def out_proj_tasks(j, TT_):
            """4 closures: out-proj chains for slab j, run as filler during
            slab j+1's attention.  ct==1 closures also evict + DMA."""
            osb = {}

            def chain(nt2, ct):
                if ct == 0:
                    osb[nt2] = outp.tile(
                        [128, 1024], f32, tag=f"o{nt2}", name=f"osb{nt2}"
                    )
                ps = ps_mm.tile([128, 512], f32, tag="pm")
                for g in range(8):
                    # permuted axis: contiguous lhsT slice
                    off = 256 * (g // 2) + 128 * nt2
                    nc.tensor.matmul(
                        ps,
                        TT_[g % 2][:, off : off + 128],
                        WT["o"][g][:, 512 * ct : 512 * (ct + 1)],
                        start=(g == 0),
                        stop=(g == 7),
                    )
                nc.scalar.copy(osb[nt2][:, 512 * ct : 512 * (ct + 1)], ps)
                if ct == 1:
                    nc.sync.dma_start(
                        out=out_d[
                            256 * j + 128 * nt2 : 256 * j + 128 * (nt2 + 1), :
                        ],
                        in_=osb[nt2],
                    )

            return [
                functools.partial(chain, nt2, ct) for nt2 in range(2) for ct in range(2)
            ]

        # prologue: slab 0 Q/K projections only (V(0) runs as attention-0
        # filler — V is first needed at PV).  K chains first: attention-0
        # needs ALL of KsT but only the first Q eviction per q'-tile.
        QsT, KsT, Vn, qk0, v0 = proj_tasks(XT0, YT0, Y160)
        for t in qk0[8:] + qk0[:8]:
            t()

        pending_v = v0  # V chains for the CURRENT slab
        pending_out = []  # out-proj filler from the previous slab
        for j in range(NSLAB):
            if j + 1 < NSLAB:
                XTn, YTn, Y16n = emit_loads(j + 1)
                Qn, Kn, Vv, qk_n, v_n = proj_tasks(XTn, YTn, Y16n)
            else:
                Qn = Kn = Vv = None
                qk_n, v_n = [], []
            if j == 0:
                for fi in range(8):
                    eng = nc.sync if fi % 2 == 0 else nc.scalar
                    eng.dma_start(
                        out=WT["o"][fi], in_=wot_in[128 * fi : 128 * (fi + 1), :]
                    )
            # filler queue: V(j), out-proj(j-1), Q/K(j+1) interleaved in 4
            # groups of (V, O, Q, K, Q, K)
            queue = []
            for i in range(4):
                if pending_v:
                    queue.append(pending_v.pop(0))
                if pending_out:
                    queue.append(pending_out.pop(0))
                queue.extend(qk_n[4 * i : 4 * (i + 1)])
            next_tasks = queue
            pending_v = v_n
            # front-loaded pops cover the pipeline ramp; the last slab has
            # only 8 filler tasks, spread 1/qt
            pops = [1] * 8 if j == NSLAB - 1 else [4, 4, 3, 3, 3, 3, 2, 2]

            # ---- attention, per q'-tile; filler interleaved ----
            # attT[p, 1024*blk + q'] = att^T[k'=128*blk+p, q']  (k'-block order;
            # blk -> (nt, tm) = (blk%2, blk//2))
            attT = atp.tile([128, 8192], fp16, tag="attT")
            attT3 = attT.rearrange("p (blk q) -> p blk q", blk=8)

            for qt in range(8):
                a = 8 * j + qt
                load_eadj(a + 2)
                eadj_t = eadj_tiles.pop(a)
                pss = ps_sc.tile([128, 1024], f32, tag="sc")
                for kh in range(2):
                    for dlo in range(2):
                        nc.tensor.matmul(
                            pss[:, 512 * kh : 512 * (kh + 1)],
                            QsT[dlo][:, 128 * qt : 128 * (qt + 1)],
                            KsT[dlo][:, 512 * kh : 512 * (kh + 1)],
                            start=(dlo == 0),
                            stop=(dlo == 1),
                        )
                exp_s = attp.tile([128, 1024], fp16, tag="exps")
                nc.scalar.activation(exp_s, pss, AF.Exp)
                attU = attp.tile([128, 1024], fp16, tag="attU")
                rsum = smalls.tile([128, 1], f32, tag="rsum")
                nc.vector.scalar_tensor_tensor(
                    out=attU,
                    in0=exp_s,
                    scalar=1.0,
                    in1=eadj_t,
                    op0=ALU.mult,
                    op1=ALU.mult,
                    accum_out=rsum,
                )
                recip = smalls.tile([128, 1], f32, tag="recip")
                nc.vector.reciprocal(recip, rsum)
                attN = attp.tile([128, 1024], fp16, tag="attN")
                nc.vector.tensor_scalar(
                    out=attN, in0=attU, scalar1=recip, scalar2=None, op0=ALU.mult
                )
                # PE filler while the softmax chain runs on ACT/DVE
                for _ in range(pops[qt]):
                    if next_tasks:
                        next_tasks.pop(0)()
                # attN -> attT: PE transposes of contiguous 128-col blocks
                # (k'-block order), then one batched PSUM->SBUF copy on DVE
                ps_at = ps_b16.tile([128, 1024], fp16, tag="pb")
                for blk in range(8):
                    src = attN[:, 128 * blk : 128 * (blk + 1)]
                    nc.tensor.transpose(ps_at[:, 128 * blk : 128 * (blk + 1)], src, ident)
                dst = attT3[:, :, 128 * qt : 128 * (qt + 1)]
                src3 = ps_at.rearrange("p (blk i) -> p blk i", blk=8)
                nc.vector.tensor_copy(dst, src3)

            # ---- PV: tempT[dlo][dv-128dlo, q'] ----
            TT_ = [tmp.tile([128, 1024], fp16, tag=f"tt{d}", name=f"tt{d}") for d in range(2)]
            for dlo in range(2):
                for qh in range(2):
                    ps = ps_mm.tile([128, 512], f32, tag="pm")
                    for blk in range(8):
                        nt, tm = blk % 2, blk // 2
                        nc.tensor.matmul(
                            ps,
                            Vn[nt][:, 256 * tm + 128 * dlo : 256 * tm + 128 * dlo + 128],
                            attT[:, 1024 * blk + 512 * qh : 1024 * blk + 512 * qh + 512],
                            start=(blk == 0),
                            stop=(blk == 7),
                        )
                    nc.scalar.copy(TT_[dlo][:, 512 * qh : 512 * (qh + 1)], ps)

            # drain any leftover filler (normally empty)
            while next_tasks:
                next_tasks.pop(0)()

            pending_out = out_proj_tasks(j, TT_)
            QsT, KsT, Vn = Qn, Kn, Vv

        # epilogue: last slab's out-projection
        for t in pending_out:
            t()

    nc.compile()
    return nc


def _get_program():
    if "nc" not in _CACHE:
        _CACHE["nc"] = _build_program()
    return _CACHE["nc"]


def _pair8(a):
    """[1024, n] -> fp8 pair-interleaved [512, 2, n]: out[128t+p, i, :] =
    a[128*(2t+i)+p, :]."""
    import ml_dtypes

    a8 = a.astype(ml_dtypes.float8_e4m3)
    n = a8.shape[1]
    return np.ascontiguousarray(
        a8.reshape(4, 2, 128, n).transpose(0, 2, 1, 3).reshape(512, 2, n)
    )


def _prep_inputs(x, y, adj, Wq, bq, Wk, bk, Wv, bv, Wo, bo):
    """Host-side prep: fp8/fp16 casts, transposes, exp(adj) with both axes
    permuted to q' = 256*(t%4) + t//4 order, per-core shards."""
    x2 = np.asarray(x, dtype=np.float32).reshape(B * T, D)
    y2 = np.asarray(y, dtype=np.float32).reshape(B * T, D)
    adj = np.asarray(adj, dtype=np.float32)

    xt32 = x2.T  # [1024, 16384]
    yt32 = y2.T
    yt16 = yt32.astype(np.float16)
    # exp(adj), both axes permuted t -> (t%4)*256 + t//4
    eadj16 = (
        np.exp(adj)
        .astype(np.float16)
        .reshape(16, 256, 4, 256, 4)
        .transpose(0, 2, 1, 4, 3)
        .reshape(16, 1024, 1024)
    )
    eadj16 = np.ascontiguousarray(eadj16)

    wq8 = _pair8(np.asarray(Wq, np.float32).T)  # unscaled; NORM folded in evict
    wk8 = _pair8(np.asarray(Wk, np.float32).T)
    wvt = np.asarray(Wv, np.float32).T.astype(np.float16)
    wot = np.asarray(Wo, np.float32).T.astype(np.float16)

    bqt = np.ascontiguousarray(np.asarray(bq, np.float32).reshape(8, 128).T)
    bkt = np.ascontiguousarray(np.asarray(bk, np.float32).reshape(8, 128).T)

    in_maps = []
    for c in range(NCORES):
        sl = slice(2048 * c, 2048 * (c + 1))
        in_maps.append(
            {
                "x8": _pair8(xt32[:, sl]),
                "y8": _pair8(yt32[:, sl]),
                "yt": np.ascontiguousarray(yt16[:, sl]),
                "eadj": eadj16[8 * (c % 2) : 8 * (c % 2) + 8],
                "wq8": wq8,
                "wk8": wk8,
                "wvt": wvt,
                "wot": wot,
                "bqt": bqt,
                "bkt": bkt,
            }
        )
    return in_maps


def kernel(x, y, adj, Wq, bq, Wk, bk, Wv, bv, Wo, bo):
    from concourse.bass_utils import run_bass_kernel_spmd

    nc = _get_program()
    in_maps = _prep_inputs(x, y, adj, Wq, bq, Wk, bk, Wv, bv, Wo, bo)
    res = run_bass_kernel_spmd(nc, in_maps, list(range(NCORES)))
    out = np.concatenate([res.results[c]["out"] for c in range(NCORES)], axis=0)
    # bv/bo fold: softmax rows sum to 1, so att@(V+bv) = att@V + bv and
    # out = dev_out + (bv @ Wo.T + bo)
    hb = np.asarray(bv, np.float32) @ np.asarray(Wo, np.float32).T + np.asarray(
        bo, np.float32
    )
    out = out + hb[None, :]
    return out.reshape(B, T, D)
